# revision 8
# baseline (speedup 1.0000x reference)
"""Trainium2 Bass kernel for the OPU (optical matmul + ADC quantize) module.

Math per r-block (j = k mod 16, 64 blocks of 16 contraction rows):
    x_c = X + vmap_lut[j, X+8],  w_c = W + wmap_lut[j, W+8]
    out = sum_r RNE16(x_c[r] @ w_c[r]),   RNE16(v) = round(v/16)*16
(ADC clip can never trigger: |mm| <= 16*8.3^2 < 2048.)

Design (v2, cost-model driven):
  - 3-term bf16 stacks per block, K=48: sx=[X; vx; X] x sw=[W; W; vw]
    -> XW + vxW + Xvw. Dropped vx@vw term has sigma ~1e-2, far below the
    per-block rounding-boundary budget (~0.18).
  - ADC quantize via PSUM-MAGIC: psum accumulators pre-set to
    MAGIC=1.5*2^27, where f32 ulp is exactly 16. Each block's single
    accumulating matmul then rounds its mm to a multiple of 16 inside the
    psum f32 add (validated bit-exact vs round(mm/16)*16 on device), so
    the 64-block quantize+sum costs zero vector-engine work.
  - LUT corrections via a 15-level is_ge telescope:
        v = A_j + sum_l B[j,l] * sign(x - c_l + 0.5)
    with Sign evaluated on the otherwise-idle Activation engine
    (per-level float bias) and the per-partition B-weighted accumulation
    on DVE (scalar_tensor_tensor); the last levels run entirely on the
    Pool/GpSimd engine as plain is_ge masks. Three engines in parallel.
  - 2D sharding (4 token-groups x 2 N-halves): TOK=512, NC=512 per core,
    which minimizes per-core elementwise LUT work (K*(BS/4 + N/2)).
  - Host prep is layout-only: exact int->bf16 casts, tiling, row
    duplication for the stack layouts. Stacks' integer parts DMA straight
    from DRAM; vx/vw placed per-chunk so matmuls pipeline behind the
    telescope.
"""
import numpy as np
from contextlib import ExitStack

import concourse.bass as bass
import concourse.bacc as bacc
import concourse.tile as tile
import concourse.mybir as mybir
from concourse import bass_utils

F32 = mybir.dt.float32
BF16 = mybir.dt.bfloat16
EQ = mybir.AluOpType.is_equal
GE = mybir.AluOpType.is_ge
MUL = mybir.AluOpType.mult
ADD = mybir.AluOpType.add
SUB = mybir.AluOpType.subtract
SIGN = mybir.ActivationFunctionType.Sign
IDENT = mybir.ActivationFunctionType.Identity
COPY = mybir.ActivationFunctionType.Copy

B, S, KDIM, N = 2, 1024, 1024, 1024
BS = B * S
NCORES = 8
AG, BG = 4, 2               # token-groups x N-halves
TOK = BS // AG              # 512 tokens per core
NC = N // BG                # 512 output cols per core
R = KDIM // 16              # 64 blocks
CH = KDIM // 128            # 8 k-chunks
MAGIC = float(3 * 2**26)    # 1.5*2^27 -> f32 ulp exactly 16
L_ACT = 11                  # telescope levels 1..L_ACT on Act+DVE
# levels L_ACT+1..15 on Pool as is_ge masks

_cache = {}


def _build():
    nc = bacc.Bacc("TRN2", target_bir_lowering=False, debug=False,
                   enable_asserts=False, num_devices=NCORES)
    xdn_d = nc.dram_tensor("xdn", [128, CH * TOK], BF16, kind="ExternalInput").ap()
    xstk_d = nc.dram_tensor("xstk", [16, R * TOK], BF16, kind="ExternalInput").ap()
    wdn_d = nc.dram_tensor("wdn", [128, CH * NC], BF16, kind="ExternalInput").ap()
    wstk_d = nc.dram_tensor("wstk", [32, R * NC], BF16, kind="ExternalInput").ap()
    vl_d = nc.dram_tensor("vlut", [128, 16], F32, kind="ExternalInput").ap()
    wl_d = nc.dram_tensor("wlut", [128, 16], F32, kind="ExternalInput").ap()
    out_d = nc.dram_tensor("out", [TOK, NC], F32, kind="ExternalOutput").ap()

    MC = TOK // 128  # 4 psum token-tiles

    with tile.TileContext(nc) as tc, ExitStack() as ctx:
        const = ctx.enter_context(tc.tile_pool(name="const", bufs=1))
        dense = ctx.enter_context(tc.tile_pool(name="dense", bufs=1))
        stk = ctx.enter_context(tc.tile_pool(name="stk", bufs=1))
        tmp = ctx.enter_context(tc.tile_pool(name="tmp", bufs=4))
        op = ctx.enter_context(tc.tile_pool(name="op", bufs=1))
        psum = ctx.enter_context(tc.tile_pool(name="psum", bufs=1, space="PSUM"))

        # --- lut-derived telescope tables (tiny [128, *] f32 tiles)
        vlutf = const.tile([128, 16], F32, tag="vlf")
        wlutf = const.tile([128, 16], F32, tag="wlf")
        nc.sync.dma_start(vlutf[:], vl_d[:, :])
        nc.sync.dma_start(wlutf[:], wl_d[:, :])
        # per-level sign biases (8.5 - l) as APs (const_aps not registered)
        lbias = const.tile([128, 16], F32, tag="lb")
        for l in range(1, 16):
            nc.vector.memset(lbias[:, l:l + 1], float(8.5 - l))
        tabs = {}
        for nm, lutf in (("v", vlutf), ("w", wlutf)):
            dlt = const.tile([128, 15], F32, tag=f"{nm}d")     # d_l - d_{l-1}
            nc.vector.tensor_sub(dlt[:], lutf[:, 1:16], lutf[:, 0:15])
            bh = const.tile([128, 15], F32, tag=f"{nm}b")      # delta/2
            nc.vector.tensor_scalar(bh[:], dlt[:], 0.5, None, op0=MUL)
            av = const.tile([128, 1], F32, tag=f"{nm}a")       # (d_0+d_L)/2
            nc.vector.tensor_add(av[:], lutf[:, 0:1], lutf[:, L_ACT:L_ACT + 1])
            nc.vector.tensor_scalar(av[:], av[:], 0.5, None, op0=MUL)
            tabs[nm] = (bh, dlt, av)

        # --- psum accumulators pre-set to MAGIC
        accs = []
        for mc in range(MC):
            acc = psum.tile([128, NC], F32, tag=f"acc{mc}")
            nc.vector.memset(acc[:], MAGIC)
            accs.append(acc)

        # --- dense tiles FIRST (the telescope's critical path), then the
        # mega-stack integer parts (only needed once matmuls start)
        xdn = dense.tile([128, CH * TOK], BF16, tag="xdn")
        nc.sync.dma_start(xdn[:], xdn_d[:, :])
        wdn = dense.tile([128, CH * NC], BF16, tag="wdn")
        nc.sync.dma_start(wdn[:], wdn_d[:, :])

        sx = stk.tile([48, R * TOK], BF16, tag="sx")
        nc.sync.dma_start(sx[0:16, :], xstk_d[:, :])
        nc.sync.dma_start(sx[32:48, :], xstk_d[:, :])
        sw = stk.tile([48, R * NC], BF16, tag="sw")
        nc.sync.dma_start(sw[0:32, :], wstk_d[:, :])

        vx = dense.tile([128, CH * TOK], BF16, tag="vx")
        vw = dense.tile([128, CH * NC], BF16, tag="vw")

        # --- per-chunk: telescope -> place -> matmuls (tile fw pipelines)
        for c in range(CH):
            for nm, src, dst, w_, sz in (("v", xdn, vx, None, TOK),
                                         ("w", wdn, vw, None, NC)):
            # (loop body below uses nm to pick tables)
                bh, dlt, av = tabs[nm]
                s = src[:, sz * c:sz * (c + 1)]
                d = dst[:, sz * c:sz * (c + 1)]
                # init: v = A_j  (Act engine, scale=0 ignores input)
                nc.scalar.activation(d[:], s[:], IDENT, bias=av[:, 0:1],
                                     scale=0.0)
                # levels 1..L_ACT: Act sign + DVE stt accumulate
                for l in range(1, L_ACT + 1):
                    m = tmp.tile([128, sz], BF16, tag=f"m{nm}{l % 2}")
                    nc.scalar.activation(m[:], s[:], SIGN, bias=lbias[:, l:l + 1])
                    nc.vector.scalar_tensor_tensor(
                        d[:], m[:], bh[:, l - 1:l], d[:], op0=MUL, op1=ADD)
                # levels L_ACT+1..15: Pool builds is_ge masks (imm scalar --
                # neuronxcc rejects AP-scalar ops on Pool); DVE accumulates.
                for l in range(L_ACT + 1, 16):
                    m = tmp.tile([128, sz], BF16, tag=f"p{nm}{l % 2}")
                    nc.gpsimd.tensor_scalar(m[:], s[:], float(l - 8.5), None,
                                            op0=GE)
                    nc.vector.scalar_tensor_tensor(
                        d[:], m[:], dlt[:, l - 1:l], d[:], op0=MUL, op1=ADD)

            # place vx (partitions 16-31 of sx) and vw (32-47 of sw)
            for m_ in range(8):
                nc.sync.dma_start(
                    sx[16:32, TOK * (8 * c + m_):TOK * (8 * c + m_ + 1)],
                    vx[16 * m_:16 * (m_ + 1), TOK * c:TOK * (c + 1)])
                nc.sync.dma_start(
                    sw[32:48, NC * (8 * c + m_):NC * (8 * c + m_ + 1)],
                    vw[16 * m_:16 * (m_ + 1), NC * c:NC * (c + 1)])

            # matmuls for this chunk's 8 blocks
            for b_ in range(8):
                r = 8 * c + b_
                for mc in range(MC):
                    nc.tensor.matmul(
                        accs[mc][:],
                        sx[:, TOK * r + 128 * mc:TOK * r + 128 * (mc + 1)],
                        sw[:, NC * r:NC * (r + 1)],
                        start=False, stop=(r == R - 1))

        # --- out = acc - MAGIC (Act engine), store
        for mc in range(MC):
            o = op.tile([128, NC], F32, tag=f"o{mc % 2}")
            nc.scalar.activation(o[:], accs[mc][:], COPY, bias=-MAGIC)
            nc.sync.dma_start(out_d[128 * mc:128 * (mc + 1), :], o[:])

    nc.compile()
    return nc


def _prep(input, weight, vmap_lut, wmap_lut):
    """Host-side pure relayout: exact int->bf16 casts, tiling, row dup."""
    import ml_dtypes
    bf = ml_dtypes.bfloat16
    x = np.asarray(input, np.float32).reshape(BS, KDIM)
    w = np.asarray(weight, np.float32)

    xT = np.ascontiguousarray(x.T).astype(bf)               # [K, BS] exact
    xdn = np.ascontiguousarray(
        xT.reshape(CH, 128, BS).transpose(1, 0, 2))         # [128, CH, BS]
    xstk = np.ascontiguousarray(
        xT.reshape(R, 16, BS).transpose(1, 0, 2))           # [16, R, BS]

    wb = w.astype(bf)                                       # exact ints
    wdn = np.ascontiguousarray(
        wb.reshape(CH, 128, N).transpose(1, 0, 2))          # [128, CH, N]
    wr = wb.reshape(R, 16, N).transpose(1, 0, 2)            # [16, R, N]
    wstk = np.ascontiguousarray(np.concatenate([wr, wr], 0))  # [32, R, N]

    vl = np.ascontiguousarray(np.tile(np.asarray(vmap_lut, np.float32), (8, 1)))
    wl = np.ascontiguousarray(np.tile(np.asarray(wmap_lut, np.float32), (8, 1)))

    in_maps = []
    for c in range(NCORES):
        a, b_ = c // BG, c % BG
        ts_ = slice(TOK * a, TOK * (a + 1))
        ns = slice(NC * b_, NC * (b_ + 1))
        in_maps.append({
            "xdn": np.ascontiguousarray(xdn[:, :, ts_]).reshape(128, CH * TOK),
            "xstk": np.ascontiguousarray(xstk[:, :, ts_]).reshape(16, R * TOK),
            "wdn": np.ascontiguousarray(wdn[:, :, ns]).reshape(128, CH * NC),
            "wstk": np.ascontiguousarray(wstk[:, :, ns]).reshape(32, R * NC),
            "vlut": vl, "wlut": wl,
        })
    return in_maps


def kernel(input, weight, vmap_lut, wmap_lut):
    if "nc" not in _cache:
        _cache["nc"] = _build()
    nc = _cache["nc"]
    in_maps = _prep(input, weight, vmap_lut, wmap_lut)
    res = bass_utils.run_bass_kernel_spmd(nc, in_maps, core_ids=list(range(NCORES)))
    out = np.zeros((BS, N), np.float32)
    for c in range(NCORES):
        a, b_ = c // BG, c % BG
        out[TOK * a:TOK * (a + 1), NC * b_:NC * (b_ + 1)] = res.results[c]["out"]
    return out.reshape(B, S, N)


# revision 13
# speedup vs baseline: 1.0219x; 1.0219x over previous
"""Trainium2 Bass kernel for the OPU (optical matmul + ADC quantize) module.

Math per r-block (j = k mod 16, 64 blocks of 16 contraction rows):
    x_c = X + vmap_lut[j, X+8],  w_c = W + wmap_lut[j, W+8]
    out = sum_r RNE16(x_c[r] @ w_c[r]),   RNE16(v) = round(v/16)*16
(ADC clip can never trigger: |mm| <= 16*8.3^2 < 2048.)

Design (v2, cost-model driven):
  - 3-term bf16 stacks per block, K=48: sx=[X; vx; X] x sw=[W; W; vw]
    -> XW + vxW + Xvw. Dropped vx@vw term has sigma ~1e-2, far below the
    per-block rounding-boundary budget (~0.18).
  - ADC quantize via PSUM-MAGIC: psum accumulators pre-set to
    MAGIC=1.5*2^27, where f32 ulp is exactly 16. Each block's single
    accumulating matmul then rounds its mm to a multiple of 16 inside the
    psum f32 add (validated bit-exact vs round(mm/16)*16 on device), so
    the 64-block quantize+sum costs zero vector-engine work.
  - LUT corrections via a 15-level is_ge telescope:
        v = A_j + sum_l B[j,l] * sign(x - c_l + 0.5)
    with Sign evaluated on the otherwise-idle Activation engine
    (per-level float bias) and the per-partition B-weighted accumulation
    on DVE (scalar_tensor_tensor); the last levels run entirely on the
    Pool/GpSimd engine as plain is_ge masks. Three engines in parallel.
  - 2D sharding (4 token-groups x 2 N-halves): TOK=512, NC=512 per core,
    which minimizes per-core elementwise LUT work (K*(BS/4 + N/2)).
  - Host prep is layout-only: exact int->bf16 casts, tiling, row
    duplication for the stack layouts. Stacks' integer parts DMA straight
    from DRAM; vx/vw placed per-chunk so matmuls pipeline behind the
    telescope.
"""
import numpy as np
from contextlib import ExitStack

import concourse.bass as bass
import concourse.bacc as bacc
import concourse.tile as tile
import concourse.mybir as mybir
from concourse import bass_utils

F32 = mybir.dt.float32
BF16 = mybir.dt.bfloat16
EQ = mybir.AluOpType.is_equal
GE = mybir.AluOpType.is_ge
MUL = mybir.AluOpType.mult
ADD = mybir.AluOpType.add
SUB = mybir.AluOpType.subtract
SIGN = mybir.ActivationFunctionType.Sign
IDENT = mybir.ActivationFunctionType.Identity
COPY = mybir.ActivationFunctionType.Copy

B, S, KDIM, N = 2, 1024, 1024, 1024
BS = B * S
NCORES = 8
AG, BG = 4, 2               # token-groups x N-halves
TOK = BS // AG              # 512 tokens per core
NC = N // BG                # 512 output cols per core
R = KDIM // 16              # 64 blocks
CH = KDIM // 128            # 8 k-chunks
MAGIC = float(3 * 2**26)    # 1.5*2^27 -> f32 ulp exactly 16
L_ACT = 11                  # telescope levels 1..L_ACT on Act+DVE
L_POOL = 14                 # levels L_POOL..15 fully on Pool (D-tile mul/add)
# levels L_ACT+1..L_POOL-1: Pool is_ge mask + DVE stt

_cache = {}


def _build():
    nc = bacc.Bacc("TRN2", target_bir_lowering=False, debug=False,
                   enable_asserts=False, num_devices=NCORES)
    xdn_d = nc.dram_tensor("xdn", [128, CH * TOK], BF16, kind="ExternalInput").ap()
    xstk_d = nc.dram_tensor("xstk", [16, R * TOK], BF16, kind="ExternalInput").ap()
    wdn_d = nc.dram_tensor("wdn", [128, CH * NC], BF16, kind="ExternalInput").ap()
    wstk_d = nc.dram_tensor("wstk", [32, R * NC], BF16, kind="ExternalInput").ap()
    vl_d = nc.dram_tensor("vlut", [128, 16], F32, kind="ExternalInput").ap()
    wl_d = nc.dram_tensor("wlut", [128, 16], F32, kind="ExternalInput").ap()
    out_d = nc.dram_tensor("out", [TOK, NC], F32, kind="ExternalOutput").ap()

    MC = TOK // 128  # 4 psum token-tiles

    with tile.TileContext(nc) as tc, ExitStack() as ctx:
        const = ctx.enter_context(tc.tile_pool(name="const", bufs=1))
        dense = ctx.enter_context(tc.tile_pool(name="dense", bufs=1))
        stk = ctx.enter_context(tc.tile_pool(name="stk", bufs=1))
        tmp = ctx.enter_context(tc.tile_pool(name="tmp", bufs=3))
        op = ctx.enter_context(tc.tile_pool(name="op", bufs=1))
        psum = ctx.enter_context(tc.tile_pool(name="psum", bufs=1, space="PSUM"))

        # --- lut-derived telescope tables (tiny [128, *] f32 tiles)
        vlutf = const.tile([128, 16], F32, tag="vlf")
        wlutf = const.tile([128, 16], F32, tag="wlf")
        nc.sync.dma_start(vlutf[:], vl_d[:, :])
        nc.sync.dma_start(wlutf[:], wl_d[:, :])
        # per-level sign biases (8.5 - l) as APs (const_aps not registered)
        lbias = const.tile([128, 16], F32, tag="lb")
        for l in range(1, 16):
            nc.vector.memset(lbias[:, l:l + 1], float(8.5 - l))
        tabs = {}
        for nm, lutf in (("v", vlutf), ("w", wlutf)):
            dlt = const.tile([128, 15], F32, tag=f"{nm}d")     # d_l - d_{l-1}
            nc.vector.tensor_sub(dlt[:], lutf[:, 1:16], lutf[:, 0:15])
            bh = const.tile([128, 15], F32, tag=f"{nm}b")      # delta/2
            nc.vector.tensor_scalar(bh[:], dlt[:], 0.5, None, op0=MUL)
            av = const.tile([128, 1], F32, tag=f"{nm}a")       # (d_0+d_L)/2
            nc.vector.tensor_add(av[:], lutf[:, 0:1], lutf[:, L_ACT:L_ACT + 1])
            nc.vector.tensor_scalar(av[:], av[:], 0.5, None, op0=MUL)
            tabs[nm] = (bh, dlt, av)

        # --- psum accumulators pre-set to MAGIC
        accs = []
        for mc in range(MC):
            acc = psum.tile([128, NC], F32, tag=f"acc{mc}")
            nc.vector.memset(acc[:], MAGIC)
            accs.append(acc)

        # --- dense tiles FIRST (the telescope's critical path), then the
        # mega-stack integer parts (only needed once matmuls start)
        xdn = dense.tile([128, CH * TOK], BF16, tag="xdn")
        nc.sync.dma_start(xdn[:], xdn_d[:, :])
        wdn = dense.tile([128, CH * NC], BF16, tag="wdn")
        nc.sync.dma_start(wdn[:], wdn_d[:, :])

        # D-tiles for the Pool-only levels: delta_l broadcast along free
        # (Act Identity with scale=0 reads any loaded [128,512] input)
        dtiles = {}
        for nm in ("v", "w"):
            for l in range(L_POOL, 16):
                dt_ = const.tile([128, 512], BF16, tag=f"D{nm}{l}")
                nc.scalar.activation(dt_[:], xdn[:, 0:512], IDENT,
                                     bias=tabs[nm][1][:, l - 1:l], scale=0.0)
                dtiles[(nm, l)] = dt_

        sx = stk.tile([48, R * TOK], BF16, tag="sx")
        nc.sync.dma_start(sx[0:16, :], xstk_d[:, :])
        nc.sync.dma_start(sx[32:48, :], xstk_d[:, :])
        sw = stk.tile([48, R * NC], BF16, tag="sw")
        nc.sync.dma_start(sw[0:32, :], wstk_d[:, :])

        vx = dense.tile([128, CH * TOK], BF16, tag="vx")
        vw = dense.tile([128, CH * NC], BF16, tag="vw")

        # --- per-chunk: telescope -> place -> matmuls (tile fw pipelines)
        for c in range(CH):
            for nm, src, dst, w_, sz in (("v", xdn, vx, None, TOK),
                                         ("w", wdn, vw, None, NC)):
            # (loop body below uses nm to pick tables)
                bh, dlt, av = tabs[nm]
                s = src[:, sz * c:sz * (c + 1)]
                d = dst[:, sz * c:sz * (c + 1)]
                # init: v = A_j  (Act engine, scale=0 ignores input)
                nc.scalar.activation(d[:], s[:], IDENT, bias=av[:, 0:1],
                                     scale=0.0)
                # levels 1..L_ACT: Act sign + DVE stt accumulate
                for l in range(1, L_ACT + 1):
                    m = tmp.tile([128, sz], BF16, tag=f"m{nm}{l % 2}")
                    nc.scalar.activation(m[:], s[:], SIGN, bias=lbias[:, l:l + 1])
                    nc.vector.scalar_tensor_tensor(
                        d[:], m[:], bh[:, l - 1:l], d[:], op0=MUL, op1=ADD)
                # levels L_ACT+1..L_POOL-1: Pool is_ge mask (imm scalar --
                # neuronxcc rejects AP-scalar ops on Pool); DVE accumulates.
                for l in range(L_ACT + 1, L_POOL):
                    m = tmp.tile([128, sz], BF16, tag=f"p{nm}{l % 2}")
                    nc.gpsimd.tensor_scalar(m[:], s[:], float(l - 8.5), None,
                                            op0=GE)
                    nc.vector.scalar_tensor_tensor(
                        d[:], m[:], dlt[:, l - 1:l], d[:], op0=MUL, op1=ADD)
                # levels L_POOL..15 fully on Pool: mask, mul by D-tile,
                # accumulate in Pool-owned d2 (first level writes, no memset)
                if L_POOL <= 15:
                    d2 = tmp.tile([128, sz], BF16, tag=f"d2{nm}")
                for l in range(L_POOL, 16):
                    m = tmp.tile([128, sz], BF16, tag=f"p{nm}{l % 2}")
                    nc.gpsimd.tensor_scalar(m[:], s[:], float(l - 8.5), None,
                                            op0=GE)
                    if l == L_POOL:
                        nc.gpsimd.tensor_mul(d2[:], m[:], dtiles[(nm, l)][:])
                    else:
                        t2 = tmp.tile([128, sz], BF16, tag=f"t2{nm}")
                        nc.gpsimd.tensor_mul(t2[:], m[:], dtiles[(nm, l)][:])
                        nc.gpsimd.tensor_add(d2[:], d2[:], t2[:])
                if L_POOL <= 15:
                    nc.vector.tensor_add(d[:], d[:], d2[:])

            # place vx (partitions 16-31 of sx) and vw (32-47 of sw)
            for m_ in range(8):
                nc.sync.dma_start(
                    sx[16:32, TOK * (8 * c + m_):TOK * (8 * c + m_ + 1)],
                    vx[16 * m_:16 * (m_ + 1), TOK * c:TOK * (c + 1)])
                nc.sync.dma_start(
                    sw[32:48, NC * (8 * c + m_):NC * (8 * c + m_ + 1)],
                    vw[16 * m_:16 * (m_ + 1), NC * c:NC * (c + 1)])

            # matmuls for this chunk's 8 blocks
            for b_ in range(8):
                r = 8 * c + b_
                for mc in range(MC):
                    nc.tensor.matmul(
                        accs[mc][:],
                        sx[:, TOK * r + 128 * mc:TOK * r + 128 * (mc + 1)],
                        sw[:, NC * r:NC * (r + 1)],
                        start=False, stop=(r == R - 1))

        # --- out = acc - MAGIC (Act engine), store
        for mc in range(MC):
            o = op.tile([128, NC], F32, tag=f"o{mc % 2}")
            nc.scalar.activation(o[:], accs[mc][:], COPY, bias=-MAGIC)
            nc.sync.dma_start(out_d[128 * mc:128 * (mc + 1), :], o[:])

    nc.compile()
    return nc


def _prep(input, weight, vmap_lut, wmap_lut):
    """Host-side pure relayout: exact int->bf16 casts, tiling, row dup."""
    import ml_dtypes
    bf = ml_dtypes.bfloat16
    x = np.asarray(input, np.float32).reshape(BS, KDIM)
    w = np.asarray(weight, np.float32)

    xT = np.ascontiguousarray(x.T).astype(bf)               # [K, BS] exact
    xdn = np.ascontiguousarray(
        xT.reshape(CH, 128, BS).transpose(1, 0, 2))         # [128, CH, BS]
    xstk = np.ascontiguousarray(
        xT.reshape(R, 16, BS).transpose(1, 0, 2))           # [16, R, BS]

    wb = w.astype(bf)                                       # exact ints
    wdn = np.ascontiguousarray(
        wb.reshape(CH, 128, N).transpose(1, 0, 2))          # [128, CH, N]
    wr = wb.reshape(R, 16, N).transpose(1, 0, 2)            # [16, R, N]
    wstk = np.ascontiguousarray(np.concatenate([wr, wr], 0))  # [32, R, N]

    vl = np.ascontiguousarray(np.tile(np.asarray(vmap_lut, np.float32), (8, 1)))
    wl = np.ascontiguousarray(np.tile(np.asarray(wmap_lut, np.float32), (8, 1)))

    in_maps = []
    for c in range(NCORES):
        a, b_ = c // BG, c % BG
        ts_ = slice(TOK * a, TOK * (a + 1))
        ns = slice(NC * b_, NC * (b_ + 1))
        in_maps.append({
            "xdn": np.ascontiguousarray(xdn[:, :, ts_]).reshape(128, CH * TOK),
            "xstk": np.ascontiguousarray(xstk[:, :, ts_]).reshape(16, R * TOK),
            "wdn": np.ascontiguousarray(wdn[:, :, ns]).reshape(128, CH * NC),
            "wstk": np.ascontiguousarray(wstk[:, :, ns]).reshape(32, R * NC),
            "vlut": vl, "wlut": wl,
        })
    return in_maps


def kernel(input, weight, vmap_lut, wmap_lut):
    if "nc" not in _cache:
        _cache["nc"] = _build()
    nc = _cache["nc"]
    in_maps = _prep(input, weight, vmap_lut, wmap_lut)
    res = bass_utils.run_bass_kernel_spmd(nc, in_maps, core_ids=list(range(NCORES)))
    out = np.zeros((BS, N), np.float32)
    for c in range(NCORES):
        a, b_ = c // BG, c % BG
        out[TOK * a:TOK * (a + 1), NC * b_:NC * (b_ + 1)] = res.results[c]["out"]
    return out.reshape(B, S, N)


# revision 14
# speedup vs baseline: 1.0426x; 1.0202x over previous
"""Trainium2 Bass kernel for the OPU (optical matmul + ADC quantize) module.

Math per r-block (j = k mod 16, 64 blocks of 16 contraction rows):
    x_c = X + vmap_lut[j, X+8],  w_c = W + wmap_lut[j, W+8]
    out = sum_r RNE16(x_c[r] @ w_c[r]),   RNE16(v) = round(v/16)*16
(ADC clip can never trigger: |mm| <= 16*8.3^2 < 2048.)

Design (v2, cost-model driven):
  - 3-term bf16 stacks per block, K=48: sx=[X; vx; X] x sw=[W; W; vw]
    -> XW + vxW + Xvw. Dropped vx@vw term has sigma ~1e-2, far below the
    per-block rounding-boundary budget (~0.18).
  - ADC quantize via PSUM-MAGIC: psum accumulators pre-set to
    MAGIC=1.5*2^27, where f32 ulp is exactly 16. Each block's single
    accumulating matmul then rounds its mm to a multiple of 16 inside the
    psum f32 add (validated bit-exact vs round(mm/16)*16 on device), so
    the 64-block quantize+sum costs zero vector-engine work.
  - LUT corrections via a 15-level is_ge telescope:
        v = A_j + sum_l B[j,l] * sign(x - c_l + 0.5)
    with Sign evaluated on the otherwise-idle Activation engine
    (per-level float bias) and the per-partition B-weighted accumulation
    on DVE (scalar_tensor_tensor); the last levels run entirely on the
    Pool/GpSimd engine as plain is_ge masks. Three engines in parallel.
  - 2D sharding (4 token-groups x 2 N-halves): TOK=512, NC=512 per core,
    which minimizes per-core elementwise LUT work (K*(BS/4 + N/2)).
  - Host prep is layout-only: exact int->bf16 casts, tiling, row
    duplication for the stack layouts. Stacks' integer parts DMA straight
    from DRAM; vx/vw placed per-chunk so matmuls pipeline behind the
    telescope.
"""
import numpy as np
from contextlib import ExitStack

import concourse.bass as bass
import concourse.bacc as bacc
import concourse.tile as tile
import concourse.mybir as mybir
from concourse import bass_utils

F32 = mybir.dt.float32
BF16 = mybir.dt.bfloat16
EQ = mybir.AluOpType.is_equal
GE = mybir.AluOpType.is_ge
MUL = mybir.AluOpType.mult
ADD = mybir.AluOpType.add
SUB = mybir.AluOpType.subtract
SIGN = mybir.ActivationFunctionType.Sign
IDENT = mybir.ActivationFunctionType.Identity
COPY = mybir.ActivationFunctionType.Copy

B, S, KDIM, N = 2, 1024, 1024, 1024
BS = B * S
NCORES = 8
AG, BG = 4, 2               # token-groups x N-halves
TOK = BS // AG              # 512 tokens per core
NC = N // BG                # 512 output cols per core
R = KDIM // 16              # 64 blocks
CH = KDIM // 128            # 8 k-chunks
MAGIC = float(3 * 2**26)    # 1.5*2^27 -> f32 ulp exactly 16
L_ACT = 11                  # telescope levels 1..L_ACT on Act+DVE
L_POOL = 14                 # levels L_POOL..15 fully on Pool (D-tile mul/add)
# levels L_ACT+1..L_POOL-1: Pool is_ge mask + DVE stt

_cache = {}


def _build():
    nc = bacc.Bacc("TRN2", target_bir_lowering=False, debug=False,
                   enable_asserts=False, num_devices=NCORES)
    xdn_d = nc.dram_tensor("xdn", [128, CH * TOK], BF16, kind="ExternalInput").ap()
    xstk_d = nc.dram_tensor("xstk", [16, R * TOK], BF16, kind="ExternalInput").ap()
    wdn_d = nc.dram_tensor("wdn", [128, CH * NC], BF16, kind="ExternalInput").ap()
    wstk_d = nc.dram_tensor("wstk", [32, R * NC], BF16, kind="ExternalInput").ap()
    vl_d = nc.dram_tensor("vlut", [128, 16], F32, kind="ExternalInput").ap()
    wl_d = nc.dram_tensor("wlut", [128, 16], F32, kind="ExternalInput").ap()
    out_d = nc.dram_tensor("out", [TOK, NC], F32, kind="ExternalOutput").ap()

    MC = TOK // 128  # 4 psum token-tiles

    with tile.TileContext(nc) as tc, ExitStack() as ctx:
        const = ctx.enter_context(tc.tile_pool(name="const", bufs=1))
        dense = ctx.enter_context(tc.tile_pool(name="dense", bufs=1))
        stk = ctx.enter_context(tc.tile_pool(name="stk", bufs=1))
        tmp = ctx.enter_context(tc.tile_pool(name="tmp", bufs=3))
        op = ctx.enter_context(tc.tile_pool(name="op", bufs=1))
        psum = ctx.enter_context(tc.tile_pool(name="psum", bufs=1, space="PSUM"))

        # --- lut-derived telescope tables (tiny [128, *] f32 tiles)
        vlutf = const.tile([128, 16], F32, tag="vlf")
        wlutf = const.tile([128, 16], F32, tag="wlf")
        nc.sync.dma_start(vlutf[:], vl_d[:, :])
        nc.sync.dma_start(wlutf[:], wl_d[:, :])
        # per-level sign biases (8.5 - l) as APs (const_aps not registered)
        lbias = const.tile([128, 16], F32, tag="lb")
        for l in range(1, 16):
            nc.vector.memset(lbias[:, l:l + 1], float(8.5 - l))
        tabs = {}
        for nm, lutf in (("v", vlutf), ("w", wlutf)):
            dlt = const.tile([128, 15], F32, tag=f"{nm}d")     # d_l - d_{l-1}
            nc.vector.tensor_sub(dlt[:], lutf[:, 1:16], lutf[:, 0:15])
            bh = const.tile([128, 15], F32, tag=f"{nm}b")      # delta/2
            nc.vector.tensor_scalar(bh[:], dlt[:], 0.5, None, op0=MUL)
            av = const.tile([128, 1], F32, tag=f"{nm}a")       # (d_0+d_L)/2
            nc.vector.tensor_add(av[:], lutf[:, 0:1], lutf[:, L_ACT:L_ACT + 1])
            nc.vector.tensor_scalar(av[:], av[:], 0.5, None, op0=MUL)
            tabs[nm] = (bh, dlt, av)

        # --- psum accumulators pre-set to MAGIC
        accs = []
        for mc in range(MC):
            acc = psum.tile([128, NC], F32, tag=f"acc{mc}")
            nc.vector.memset(acc[:], MAGIC)
            accs.append(acc)

        # --- dense tiles FIRST (the telescope's critical path), then the
        # mega-stack integer parts (only needed once matmuls start)
        xdn = dense.tile([128, CH * TOK], BF16, tag="xdn")
        nc.sync.dma_start(xdn[:], xdn_d[:, :])
        wdn = dense.tile([128, CH * NC], BF16, tag="wdn")
        nc.sync.dma_start(wdn[:], wdn_d[:, :])

        # D-tiles for the Pool-only levels: delta_l broadcast along free
        # (Act Identity with scale=0 reads any loaded [128,512] input)
        dtiles = {}
        for nm in ("v", "w"):
            for l in range(L_POOL, 16):
                dt_ = const.tile([128, 512], BF16, tag=f"D{nm}{l}")
                nc.scalar.activation(dt_[:], xdn[:, 0:512], IDENT,
                                     bias=tabs[nm][1][:, l - 1:l], scale=0.0)
                dtiles[(nm, l)] = dt_

        sx = stk.tile([48, R * TOK], BF16, tag="sx")
        nc.sync.dma_start(sx[0:16, :], xstk_d[:, :])
        nc.sync.dma_start(sx[32:48, :], xstk_d[:, :])
        sw = stk.tile([48, R * NC], BF16, tag="sw")
        nc.sync.dma_start(sw[0:32, :], wstk_d[:, :])

        vx = dense.tile([128, CH * TOK], BF16, tag="vx")
        vw = dense.tile([128, CH * NC], BF16, tag="vw")

        # --- per-chunk: telescope -> place -> matmuls (tile fw pipelines)
        for c in range(CH):
            for nm, src, dst, w_, sz in (("v", xdn, vx, None, TOK),
                                         ("w", wdn, vw, None, NC)):
            # (loop body below uses nm to pick tables)
                bh, dlt, av = tabs[nm]
                s = src[:, sz * c:sz * (c + 1)]
                d = dst[:, sz * c:sz * (c + 1)]
                # init: v = A_j  (Act engine, scale=0 ignores input)
                nc.scalar.activation(d[:], s[:], IDENT, bias=av[:, 0:1],
                                     scale=0.0)
                # levels 1..L_ACT: Act sign + DVE stt accumulate
                for l in range(1, L_ACT + 1):
                    m = tmp.tile([128, sz], BF16, tag=f"m{nm}{l % 2}")
                    nc.scalar.activation(m[:], s[:], SIGN, bias=lbias[:, l:l + 1])
                    nc.vector.scalar_tensor_tensor(
                        d[:], m[:], bh[:, l - 1:l], d[:], op0=MUL, op1=ADD)
                # levels L_ACT+1..L_POOL-1: Pool is_ge mask (imm scalar --
                # neuronxcc rejects AP-scalar ops on Pool); DVE accumulates.
                for l in range(L_ACT + 1, L_POOL):
                    m = tmp.tile([128, sz], BF16, tag=f"p{nm}{l % 2}")
                    nc.gpsimd.tensor_scalar(m[:], s[:], float(l - 8.5), None,
                                            op0=GE)
                    nc.vector.scalar_tensor_tensor(
                        d[:], m[:], dlt[:, l - 1:l], d[:], op0=MUL, op1=ADD)
                # levels L_POOL..15 fully on Pool: mask, mul by D-tile,
                # accumulate in Pool-owned d2 (first level writes, no memset)
                if L_POOL <= 15:
                    d2 = tmp.tile([128, sz], BF16, tag=f"d2{nm}")
                for l in range(L_POOL, 16):
                    m = tmp.tile([128, sz], BF16, tag=f"p{nm}{l % 2}")
                    nc.gpsimd.tensor_scalar(m[:], s[:], float(l - 8.5), None,
                                            op0=GE)
                    if l == L_POOL:
                        nc.gpsimd.tensor_mul(d2[:], m[:], dtiles[(nm, l)][:])
                    else:
                        t2 = tmp.tile([128, sz], BF16, tag=f"t2{nm}")
                        nc.gpsimd.tensor_mul(t2[:], m[:], dtiles[(nm, l)][:])
                        nc.gpsimd.tensor_add(d2[:], d2[:], t2[:])
                if L_POOL <= 15:
                    nc.gpsimd.tensor_add(d[:], d[:], d2[:])

            # place vx (partitions 16-31 of sx) and vw (32-47 of sw)
            for m_ in range(8):
                nc.sync.dma_start(
                    sx[16:32, TOK * (8 * c + m_):TOK * (8 * c + m_ + 1)],
                    vx[16 * m_:16 * (m_ + 1), TOK * c:TOK * (c + 1)])
                nc.sync.dma_start(
                    sw[32:48, NC * (8 * c + m_):NC * (8 * c + m_ + 1)],
                    vw[16 * m_:16 * (m_ + 1), NC * c:NC * (c + 1)])

            # matmuls for this chunk's 8 blocks
            for b_ in range(8):
                r = 8 * c + b_
                for mc in range(MC):
                    nc.tensor.matmul(
                        accs[mc][:],
                        sx[:, TOK * r + 128 * mc:TOK * r + 128 * (mc + 1)],
                        sw[:, NC * r:NC * (r + 1)],
                        start=False, stop=(r == R - 1))

        # --- out = acc - MAGIC (Act engine), store
        for mc in range(MC):
            o = op.tile([128, NC], F32, tag=f"o{mc % 2}")
            nc.scalar.activation(o[:], accs[mc][:], COPY, bias=-MAGIC)
            nc.sync.dma_start(out_d[128 * mc:128 * (mc + 1), :], o[:])

    nc.compile()
    return nc


def _prep(input, weight, vmap_lut, wmap_lut):
    """Host-side pure relayout: exact int->bf16 casts, tiling, row dup."""
    import ml_dtypes
    bf = ml_dtypes.bfloat16
    x = np.asarray(input, np.float32).reshape(BS, KDIM)
    w = np.asarray(weight, np.float32)

    xT = np.ascontiguousarray(x.T).astype(bf)               # [K, BS] exact
    xdn = np.ascontiguousarray(
        xT.reshape(CH, 128, BS).transpose(1, 0, 2))         # [128, CH, BS]
    xstk = np.ascontiguousarray(
        xT.reshape(R, 16, BS).transpose(1, 0, 2))           # [16, R, BS]

    wb = w.astype(bf)                                       # exact ints
    wdn = np.ascontiguousarray(
        wb.reshape(CH, 128, N).transpose(1, 0, 2))          # [128, CH, N]
    wr = wb.reshape(R, 16, N).transpose(1, 0, 2)            # [16, R, N]
    wstk = np.ascontiguousarray(np.concatenate([wr, wr], 0))  # [32, R, N]

    vl = np.ascontiguousarray(np.tile(np.asarray(vmap_lut, np.float32), (8, 1)))
    wl = np.ascontiguousarray(np.tile(np.asarray(wmap_lut, np.float32), (8, 1)))

    in_maps = []
    for c in range(NCORES):
        a, b_ = c // BG, c % BG
        ts_ = slice(TOK * a, TOK * (a + 1))
        ns = slice(NC * b_, NC * (b_ + 1))
        in_maps.append({
            "xdn": np.ascontiguousarray(xdn[:, :, ts_]).reshape(128, CH * TOK),
            "xstk": np.ascontiguousarray(xstk[:, :, ts_]).reshape(16, R * TOK),
            "wdn": np.ascontiguousarray(wdn[:, :, ns]).reshape(128, CH * NC),
            "wstk": np.ascontiguousarray(wstk[:, :, ns]).reshape(32, R * NC),
            "vlut": vl, "wlut": wl,
        })
    return in_maps


def kernel(input, weight, vmap_lut, wmap_lut):
    if "nc" not in _cache:
        _cache["nc"] = _build()
    nc = _cache["nc"]
    in_maps = _prep(input, weight, vmap_lut, wmap_lut)
    res = bass_utils.run_bass_kernel_spmd(nc, in_maps, core_ids=list(range(NCORES)))
    out = np.zeros((BS, N), np.float32)
    for c in range(NCORES):
        a, b_ = c // BG, c % BG
        out[TOK * a:TOK * (a + 1), NC * b_:NC * (b_ + 1)] = res.results[c]["out"]
    return out.reshape(B, S, N)
